# revision 58
# baseline (speedup 1.0000x reference)
"""CRF negative log-likelihood on 8 Trainium2 NeuronCores.

Problem: B=128, T=2048, K=96 linear-chain CRF loss (log-partition via the
forward algorithm minus the joint path score), mask is all-ones.

Strategy
--------
Batch dim B is sharded 16 sequences per core (data parallel); the [K,K]
transition matrix is replicated. Per core:

* log-partition: the logsumexp forward recurrence is rewritten in the exp
  domain:  p_t = (p_{t-1} @ E) * xhat_t  with  E = exp(transitions) and
  xhat_t = exp(emit_t - C0).  Each step is one tiny PE matmul
  ([96x96] @ [96,16]) plus one DVE elementwise multiply.  The constant
  per-step offset C0 keeps p_t in fp32/bf16 exponent range over the whole
  chain (ln max|p| stays within [-4, 24] for standard-normal logits; the
  exact log-magnitude drift was validated offline), so the log is taken
  only once at the end:  logZ = log(sum_j p_last * exp(end)) + (T-1)*C0.
  To halve the serial latency chain, a forward recurrence from t=0 and a
  backward recurrence from t=T-1 run interleaved and meet in the middle:
  Z = p_{t*}^T E w_{t*+1}.
* joint score: one-hot label tiles ([t,K] layout, DVE compare against an
  iota) give the emission score via a fused multiply-reduce
  (scalar_tensor_tensor with accum_out), and the transition score via PE
  pair-count matmuls (count[i,j] = #(t: l_t=i, l_{t+1}=j)) followed by a
  fused <count, transitions> reduce. xhat production (DMA -> ACT exp ->
  PE transpose -> ACT copy) runs PRE=64 quanta ahead of the chains with
  no DVE work, and the DVE-heavy score parts lag by the same amount, so
  the serial chain — which saturates the DVE — starts immediately and is
  never starved.

Each core returns a small vector of partial sums; the host only assembles
the final scalar: loss = -sum_b (score_b - logZ_b).
"""
import sys

sys.path.insert(0, "/opt/trn_rl_repo")

import numpy as np

import concourse.bacc as bacc
import concourse.bass as bass
import concourse.mybir as mybir
from concourse.bass_utils import run_bass_kernel_spmd
from concourse.tile import TileContext

import os

_DISABLE = set(os.environ.get("CRF_DISABLE", "").split(","))

B, T, K = 128, 2048, 96
N_CORES = 8
BL = B // N_CORES          # 16 sequences per core
C0 = 5.06                  # per-step scale offset, ~E[log growth]
CHUNK = 128                # time-steps per prefetch/score chunk
F32 = mybir.dt.float32
BF16 = mybir.dt.bfloat16
I32 = mybir.dt.int32
EXP = mybir.ActivationFunctionType.Exp
MULT = mybir.AluOpType.mult
ADD = mybir.AluOpType.add
EQ = mybir.AluOpType.is_equal


def build_program(t_total=T, bl=BL, chunk=CHUNK):
    nchunk = t_total // chunk
    tstar = t_total // 2 - 1           # forward chain covers t=1..tstar
    ntile = nchunk                     # score tiles per sequence

    nc = bacc.Bacc(None, target_bir_lowering=False)
    lg_in = nc.declare_dram_parameter("logits", [bl, t_total, K], F32, isOutput=False)
    lab_in = nc.declare_dram_parameter("labels", [bl, t_total], F32, isOutput=False)
    labn_in = nc.declare_dram_parameter("labels_next", [bl, t_total], F32, isOutput=False)
    tr_in = nc.declare_dram_parameter("transitions", [K, K], F32, isOutput=False)
    st_in = nc.declare_dram_parameter("start_t", [K, 1], F32, isOutput=False)
    en_in = nc.declare_dram_parameter("end_t", [K, 1], F32, isOutput=False)
    # first and last label of each sequence (host-sliced), for the start/end
    # transition scores — avoids 1-partition ops, which fault on HW
    le_in = nc.declare_dram_parameter("lab_edge", [2, bl], F32, isOutput=False)
    # stackA column layout: [0:bl] zpre, [2bl:3bl] start, [3bl:4bl] end,
    # [4bl : 4bl+ntile*bl] per-(b,tile) transition-score partials
    a_w = 4 * bl + ntile * bl
    out_w = a_w + ntile * bl                       # + emit partials
    y_out = nc.declare_dram_parameter("y", [1, out_w], F32, isOutput=True)

    with TileContext(nc) as tc:
        with (
            tc.tile_pool(name="const", bufs=1) as cpool,
            tc.tile_pool(name="xh", bufs=1) as xpool,
            tc.tile_pool(name="state", bufs=3) as stpool,
            tc.tile_pool(name="score", bufs=5) as scpool,
            tc.tile_pool(name="scratch", bufs=2) as scrpool,
            tc.tile_pool(name="stacks", bufs=1) as kpool,
            tc.tile_pool(name="ps_f", bufs=2, space="PSUM") as psf,
            tc.tile_pool(name="ps_b", bufs=2, space="PSUM") as psb,
            tc.tile_pool(name="ps_x", bufs=1, space="PSUM") as psx_pool,
            tc.tile_pool(name="ps_c", bufs=1, space="PSUM") as psc,
            tc.tile_pool(name="ps_o", bufs=1, space="PSUM") as pso,
        ):
            # ---- constants -------------------------------------------------
            tr_f = cpool.tile([K, K], F32, tag="tr_f")
            trT_f = cpool.tile([K, K], F32, tag="trT_f")
            nc.sync.dma_start(out=tr_f[:], in_=tr_in[:])
            nc.sync.dma_start(out=trT_f[:], in_=tr_in[:].rearrange("i j -> j i"))
            e_sb = cpool.tile([K, K], BF16, tag="e_sb")
            eT_sb = cpool.tile([K, K], BF16, tag="eT_sb")
            nc.scalar.activation(e_sb[:], tr_f[:], EXP)
            nc.scalar.activation(eT_sb[:], trT_f[:], EXP)

            st_col = cpool.tile([K, 1], F32, tag="st_col")
            en_col = cpool.tile([K, 1], F32, tag="en_col")
            nc.sync.dma_start(out=st_col[:], in_=st_in[:])
            nc.sync.dma_start(out=en_col[:], in_=en_in[:])
            een_col = cpool.tile([K, 1], F32, tag="een_col")
            nc.scalar.activation(een_col[:], en_col[:], EXP)
            # first/last labels broadcast down all K partitions
            labs0 = cpool.tile([K, bl], F32, tag="labs0")
            labs1 = cpool.tile([K, bl], F32, tag="labs1")
            nc.sync.dma_start(out=labs0[:], in_=le_in[0:1, :].to_broadcast([K, bl]))
            nc.sync.dma_start(out=labs1[:], in_=le_in[1:2, :].to_broadcast([K, bl]))
            iotac_i = cpool.tile([K, 1], I32, tag="iotac_i")
            nc.gpsimd.iota(iotac_i[:], pattern=[[1, 1]], base=0, channel_multiplier=1)
            iotac = cpool.tile([K, 1], F32, tag="iotac")
            nc.vector.tensor_copy(iotac[:], iotac_i[:])

            negc0 = cpool.tile([chunk, 1], F32, tag="negc0")
            nc.vector.memset(negc0[:], -C0)
            posc0 = cpool.tile([K, 1], F32, tag="posc0")
            nc.vector.memset(posc0[:], C0)
            # exp(start + C0) column, used to seed p0 from xhat_0
            stc0 = cpool.tile([K, 1], F32, tag="stc0")
            nc.scalar.activation(stc0[:], st_col[:], EXP, bias=posc0[:])
            # identity for PE transposes
            ones2d = cpool.tile([chunk, chunk], BF16, tag="ones2d")
            nc.vector.memset(ones2d[:], 1.0)
            ident = cpool.tile([chunk, chunk], BF16, tag="ident")
            nc.gpsimd.affine_select(
                ident[:], ones2d[:], pattern=[[1, chunk]],
                compare_op=EQ, fill=0.0, base=0, channel_multiplier=-1,
            )
            iota_i = cpool.tile([chunk, K], I32, tag="iota_i")
            nc.gpsimd.iota(iota_i[:], pattern=[[1, K]], base=0, channel_multiplier=0)
            # bf16 iota: values 0..95 are exact, and 16-bit input gets the
            # DVE 2X mode for the 512 one-hot compares
            iota = cpool.tile([chunk, K], BF16, tag="iota")
            nc.vector.tensor_copy(iota[:], iota_i[:])
            ones96 = cpool.tile([K, 1], F32, tag="ones96")
            ones128 = cpool.tile([chunk, 1], F32, tag="ones128")
            nc.vector.memset(ones96[:], 1.0)
            nc.vector.memset(ones128[:], 1.0)

            lab_sb = []
            labn_sb = []
            for b in range(bl):
                lt = cpool.tile([chunk, ntile], F32, tag=f"lab{b}")
                nc.sync.dma_start(
                    out=lt[:],
                    in_=lab_in[b : b + 1, :].rearrange("o (c t) -> (o t) c", t=chunk),
                )
                lab_sb.append(lt)
                ln = cpool.tile([chunk, ntile], F32, tag=f"labn{b}")
                nc.sync.dma_start(
                    out=ln[:],
                    in_=labn_in[b : b + 1, :].rearrange("o (c t) -> (o t) c", t=chunk),
                )
                labn_sb.append(ln)

            stackA = kpool.tile([K, a_w], F32, tag="stackA")
            stackB = kpool.tile([chunk, ntile * bl], F32, tag="stackB")
            outstage = kpool.tile([1, out_w], F32, tag="outstage")
            nc.vector.memset(stackA[:], 0.0)
            if _DISABLE & {"score", "chain"}:
                nc.vector.memset(stackB[:], 0.0)
                nc.vector.memset(outstage[:], 0.0)

            # ---- combined xhat-production + score quanta -------------------
            # Tile-major order alternating from both ends of the sequence so
            # the forward and backward chains both have their xhat ready.
            tile_order = []
            for i in range((ntile + 1) // 2):
                tile_order.append(i)
                j = ntile - 1 - i
                if j != i:
                    tile_order.append(j)
            xh = [None] * nchunk
            pending_cnt = []

            em_tiles = {}

            def xh_part(q):
                tile = tile_order[q // bl]
                b = q % bl
                t0 = tile * chunk
                em = scpool.tile(
                    [chunk, K], F32, tag=f"em{q % 72}", bufs=1, name=f"em{q}"
                )
                nc.sync.dma_start(out=em[:], in_=lg_in[b, t0 : t0 + chunk, :])
                em_tiles[q] = em
                # xhat production: exp -> PE transpose -> [K, b, t] storage.
                # No DVE work here, so the startup burst does not delay the
                # first chain steps waiting in the DVE queue.
                etile = scpool.tile([chunk, K], BF16, tag="etile")
                nc.scalar.activation(etile[:], em[:], EXP, bias=negc0[:])
                if b == 0:
                    xh[tile] = xpool.tile(
                        [K, bl, chunk], BF16, tag=f"xh{tile}", name=f"xh{tile}"
                    )
                psx = psx_pool.tile([K, chunk], BF16, tag="psx")
                nc.tensor.transpose(psx[:], etile[:], ident[:])
                nc.scalar.activation(
                    xh[tile][:, b, :], psx[:], mybir.ActivationFunctionType.Copy
                )

            def score_part(q):
                if "score" in _DISABLE:
                    em_tiles.pop(q)
                    return
                tile = tile_order[q // bl]
                b = q % bl
                em = em_tiles.pop(q)
                # one-hot label tiles (DVE; gpsimd's software tensor_scalar
                # costs ~1.7us per tile and stalls the pipeline)
                ohc = scpool.tile([chunk, K], BF16, tag="ohc")
                ohn = scpool.tile([chunk, K], BF16, tag="ohn")
                nc.vector.tensor_scalar(
                    ohc[:], iota[:], lab_sb[b][:, tile : tile + 1], None, op0=EQ
                )
                nc.vector.tensor_scalar(
                    ohn[:], iota[:], labn_sb[b][:, tile : tile + 1], None, op0=EQ
                )
                col = b * ntile + tile
                if "semit" not in _DISABLE:
                    # emission score partial: sum_j onehot * logits, per t-row
                    # (scalar_tensor_tensor: out = (in0*1.0)*in1, accum = row
                    # sum; tensor_tensor_reduce faults the DVE on real TRN2)
                    scr = scrpool.tile([chunk, K], BF16, tag="scr")
                    nc.vector.scalar_tensor_tensor(
                        out=scr[:],
                        in0=ohc[:],
                        scalar=1.0,
                        in1=em[:],
                        op0=MULT,
                        op1=MULT,
                        accum_out=stackB[:, col : col + 1],
                    )
                if "scnt" not in _DISABLE:
                    pending_cnt.append((ohc, ohn, tile, col))

            def drain_cnt():
                # transition score partial: <ohc^T ohn, transitions> via PE
                # pair-count matmul then a fused multiply-reduce into stackA.
                # Deferred a few quanta so the GPSIMD-built ohn is ready well
                # before the PE matmul asks for it.
                ohc, ohn, tile, col = pending_cnt.pop(0)
                cps = psc.tile([K, K], F32, tag="cps")
                rows = chunk if tile < ntile - 1 else chunk - 1
                nc.tensor.matmul(
                    cps[:], ohc[0:rows, :], ohn[0:rows, :], start=True, stop=True
                )
                scr3 = scrpool.tile([K, K], F32, tag="scr3")
                nc.vector.scalar_tensor_tensor(
                    out=scr3[:],
                    in0=cps[:],
                    scalar=1.0,
                    in1=tr_f[:],
                    op0=MULT,
                    op1=MULT,
                    accum_out=stackA[:, 4 * bl + col : 4 * bl + col + 1],
                )

            def edge_scores():
                # start/end transition scores via one-hot columns folded into
                # the stackA partition-sum: stackA[j, 2bl+b] = start[j] *
                # (j == labels[b,0]), and likewise for end at cols 3bl..4bl.
                oh0 = scrpool.tile([K, bl], BF16, tag="oh0")
                nc.vector.tensor_scalar(oh0[:], labs0[:], iotac[:], None, op0=EQ)
                nc.vector.tensor_scalar_mul(
                    stackA[:, 2 * bl : 3 * bl], oh0[:], st_col[:]
                )
                oh1 = scrpool.tile([K, bl], BF16, tag="oh1")
                nc.vector.tensor_scalar(oh1[:], labs1[:], iotac[:], None, op0=EQ)
                nc.vector.tensor_scalar_mul(
                    stackA[:, 3 * bl : 4 * bl], oh1[:], en_col[:]
                )

            # ---- the two recurrence chains, interleaved with quanta --------
            # xh production runs PRE quanta ahead; score parts lag by PRE so
            # the DVE queue opens with chain steps, not score bursts.
            n_quanta = bl * ntile
            pending_cnt_delay = 3
            pre = min(4 * bl, n_quanta)
            for q in range(pre):
                xh_part(q)

            # p0 = exp(start + emit_0) = xhat_0 * exp(start + C0)
            p0 = stpool.tile([K, bl], BF16, tag="p0")
            nc.vector.tensor_scalar_mul(p0[:], xh[0][:, :, 0], stc0[:])
            # w_{T-1} = xhat_{T-1} * exp(end)
            w0 = stpool.tile([K, bl], BF16, tag="w0")
            nc.vector.tensor_scalar_mul(
                w0[:], xh[nchunk - 1][:, :, chunk - 1], een_col[:]
            )

            nsteps = tstar                      # fwd t=1..tstar, bwd t=T-2..tstar+1
            if "chain" in _DISABLE:
                nsteps = 8
            qx = pre
            qs = 0
            p_cur, w_cur = p0, w0
            for i in range(nsteps):
                t = 1 + i
                q_ps = psf.tile([K, bl], F32, tag="qf")
                nc.tensor.matmul(q_ps[:], e_sb[:], p_cur[:], start=True, stop=True)
                p_new = stpool.tile([K, bl], BF16, tag="p")
                c, ti = divmod(t, chunk)
                nc.vector.tensor_mul(p_new[:], q_ps[:], xh[c][:, :, ti])
                p_cur = p_new
                t = t_total - 2 - i
                u_ps = psb.tile([K, bl], F32, tag="ub")
                nc.tensor.matmul(u_ps[:], eT_sb[:], w_cur[:], start=True, stop=True)
                w_new = stpool.tile([K, bl], BF16, tag="w")
                c, ti = divmod(t, chunk)
                nc.vector.tensor_mul(w_new[:], u_ps[:], xh[c][:, :, ti])
                w_cur = w_new
                if i % 4 == 0:
                    if qx < n_quanta:
                        xh_part(qx)
                        qx += 1
                    if qs < n_quanta:
                        score_part(qs)
                        while len(pending_cnt) > pending_cnt_delay:
                            drain_cnt()
                        qs += 1
            while qx < n_quanta:
                xh_part(qx)
                qx += 1
            while qs < n_quanta:
                score_part(qs)
                while len(pending_cnt) > pending_cnt_delay:
                    drain_cnt()
                qs += 1
            while pending_cnt:
                drain_cnt()
            if "score" not in _DISABLE:
                edge_scores()

            # ---- combine: Z = p_{t*}^T E w_{t*+1} --------------------------
            qz = psf.tile([K, bl], F32, tag="qf")
            nc.tensor.matmul(qz[:], e_sb[:], p_cur[:], start=True, stop=True)
            nc.vector.tensor_mul(stackA[:, 0:bl], qz[:], w_cur[:])

            # ---- partition sums via ones-matmuls ---------------------------
            # each matmul output must stay inside one 2KB PSUM bank, so the
            # second sum starts at the 512-float bank boundary
            fin = pso.tile([1, 1024], F32, tag="fin")
            nc.tensor.matmul(
                fin[:, 0:a_w], ones96[:], stackA[:], start=True, stop=True
            )
            nc.tensor.matmul(
                fin[:, 512 : 512 + ntile * bl], ones128[:], stackB[:],
                start=True, stop=True,
            )
            nc.vector.tensor_copy(outstage[:, 0:a_w], fin[:, 0:a_w])
            nc.vector.tensor_copy(
                outstage[:, a_w:], fin[:, 512 : 512 + ntile * bl]
            )
            nc.sync.dma_start(out=y_out[:], in_=outstage[:])

    nc.compile()
    return nc


_cached = {}


def _get_program(t_total=T, bl=BL, chunk=CHUNK):
    key = (t_total, bl, chunk)
    if key not in _cached:
        _cached[key] = build_program(t_total, bl, chunk)
    return _cached[key]


def host_combine(y_rows, t_total=T, bl=BL, chunk=CHUNK):
    """Combine per-core output rows into the scalar loss."""
    ntile = t_total // chunk
    a_w = 4 * bl + ntile * bl
    total = 0.0
    for v in y_rows:
        v = np.asarray(v, np.float64).reshape(-1)
        zpre = v[0:bl]
        trans_s = v[4 * bl : a_w].reshape(bl, ntile).sum(axis=1)
        emit_s = v[a_w:].reshape(bl, ntile).sum(axis=1)
        start_s = v[2 * bl : 3 * bl]
        end_s = v[3 * bl : 4 * bl]
        logz = np.log(zpre) + (t_total - 1) * C0
        score = emit_s + trans_s + start_s + end_s
        total += (score - logz).sum()
    return np.float32(-total)


def kernel(logits, labels, mask, transitions, start_transitions, end_transitions):
    # mask is all-ones for this problem (spec fill=ones); it does not enter
    # the computation.
    logits = np.ascontiguousarray(logits, np.float32)
    labels = np.ascontiguousarray(labels).astype(np.float32)
    labels_next = np.concatenate([labels[:, 1:], labels[:, -1:]], axis=1)
    lab_edge = np.stack([labels[:, 0], labels[:, -1]])
    transitions = np.ascontiguousarray(transitions, np.float32)
    start_t = np.ascontiguousarray(start_transitions, np.float32)
    end_t = np.ascontiguousarray(end_transitions, np.float32)

    nc = _get_program()
    in_maps = []
    for c in range(N_CORES):
        sl = slice(c * BL, (c + 1) * BL)
        in_maps.append(
            {
                "logits": logits[sl],
                "labels": np.ascontiguousarray(labels[sl]),
                "labels_next": np.ascontiguousarray(labels_next[sl]),
                "transitions": transitions,
                "start_t": start_t.reshape(K, 1),
                "end_t": end_t.reshape(K, 1),
                "lab_edge": np.ascontiguousarray(lab_edge[:, sl]),
            }
        )
    res = run_bass_kernel_spmd(nc, in_maps, core_ids=list(range(N_CORES)))
    return host_combine([res.results[c]["y"] for c in range(N_CORES)])
